# revision 52
# baseline (speedup 1.0000x reference)
"""Trainium2 Bass kernel for nn_BiLSTMLag1 (4-layer BiLSTM + FC head).

Strategy (8 NeuronCores, SPMD):
  - Shard: 4 time-chunks (256 steps) x 2 batch-halves (512 samples).
  - Time-sharding is exact-to-fp32 via truncated warmup (W steps): LSTM state
    influence decays ~0.5^k, so a chain started W steps early from zero state
    matches the full scan below the bf16 noise floor. Out-of-range warmup
    steps are masked by a per-timestep "ones row" (bias gate) + zero inputs,
    which keeps the state exactly zero, matching the reference init.
  - Layer cascade: layer l's valid output region extends (4-l)*W beyond the
    core's chunk so the next layer's warmup reads locally-computed data.
  - Inverted layout: batch (128) on partitions, gates on the free dim.
    Per step, per pair of chains (fwd+bwd), all 4 batch-tiles share one
    activation instruction. Recurrent matmul uses the data as the stationary
    operand: lhsT = [x_t; 1; h_f; h_b; x_b; 1] stacked rows, rhs = a
    block-diagonal weight matrix, giving gates for both chains of one batch
    tile in a single matmul.
  - Gates use sigmoid only (tanh(g) = 2*sigmoid(2g)-1 with g-rows prescaled
    by 2 host-side); per-step h returns to matmul orientation via a PE
    transpose + one DVE copy.
"""

import numpy as np
import ml_dtypes

import concourse.bass as bass
import concourse.mybir as mybir
from concourse import bacc
from concourse.tile import TileContext
from concourse.masks import make_identity

BF16 = ml_dtypes.bfloat16
FP32 = mybir.dt.float32
BF = mybir.dt.bfloat16
AF = mybir.ActivationFunctionType
ALU = mybir.AluOpType

# layer dims: (din, H)
LAYERS = [(16, 20), (40, 20), (40, 10), (20, 10)]

# -------- configuration --------
class Cfg:
    def __init__(self, T=1024, B=1024, W=16, K=4, n_cores=8, reps=1):
        self.T, self.B, self.W, self.K = T, B, W, K
        self.n_cores = n_cores
        self.reps = reps
        self.n_tc = 4                    # time chunks
        self.n_bh = n_cores // self.n_tc  # batch halves
        self.chunk = T // self.n_tc
        self.CB = B // self.n_bh         # batch per core
        self.NBT = self.CB // 128        # 128-row batch tiles per core
        self.TS = self.chunk + 8 * W     # local time-span of all buffers

    def valid(self, l):  # local [v0, v1) of layer l's output region
        return (l * self.W, self.TS - l * self.W)


def _gate_perm(H):
    # torch gate order i,f,g,o -> ours i,f,o,g
    return np.concatenate([np.arange(0, H), np.arange(H, 2 * H),
                           np.arange(3 * H, 4 * H), np.arange(2 * H, 3 * H)])


def _rowmap(l):
    # x-staging tile rows (the h state lives in a separate tile; the gate
    # matmul is split into an x-part and an h-part accumulating into PSUM)
    din, H = LAYERS[l]
    xf0 = 0
    onesf = din
    xb0 = din + 1
    onesb = xb0 + din
    ktot = onesb + 1          # = 2*din + 2
    return xf0, onesf, xb0, onesb, ktot


def _prep_weights(inputs):
    """Per layer: wx[l] [2din+2, 2G] (x+bias block-diag moving operand) and
    wh[l] [2H, 2G] (recurrent block-diag). Gate cols reordered i,f,o,g,
    g-cols prescaled x2, bias folded in the ones rows. Layer 4 packs fwd
    weights in both halves (fwd-only pairs). Also w4bx [21, 40] for the
    single backward step of layer 4 (h=0 there, so no h-part)."""
    wxs, whs = [], []
    for l, (din, H) in enumerate(LAYERS):
        G = 4 * H
        perm = _gate_perm(H)
        xf0, onesf, xb0, onesb, ktot = _rowmap(l)
        mx = np.zeros((ktot, 2 * G), np.float32)
        mh = np.zeros((2 * H, 2 * G), np.float32)
        li = l + 1
        for half in range(2):
            dr = "f" if (half == 0 or l == 3) else "b"
            wi = inputs[f"w{li}{dr}_ih"].astype(np.float32)[perm].T  # [din, G]
            wh = inputs[f"w{li}{dr}_hh"].astype(np.float32)[perm].T  # [H, G]
            b = (inputs[f"b{li}{dr}_ih"] + inputs[f"b{li}{dr}_hh"]).astype(np.float32)[perm]
            wi = wi.copy(); wh = wh.copy(); b = b.copy()
            wi[:, 3 * H:] *= 2.0; wh[:, 3 * H:] *= 2.0; b[3 * H:] *= 2.0
            c0 = half * G
            x0 = xf0 if half == 0 else xb0
            o0 = onesf if half == 0 else onesb
            mx[x0:x0 + din, c0:c0 + G] = wi
            mx[o0, c0:c0 + G] = b
            mh[half * H:(half + 1) * H, c0:c0 + G] = wh
        wxs.append(mx.astype(BF16))
        whs.append(mh.astype(BF16))
    # layer-4 backward single step: rows [x(20); ones]
    din, H = LAYERS[3]
    G = 4 * H
    perm = _gate_perm(H)
    wi = inputs["w4b_ih"].astype(np.float32)[perm].T.copy()
    b = (inputs["b4b_ih"] + inputs["b4b_hh"]).astype(np.float32)[perm].copy()
    wi[:, 3 * H:] *= 2.0; b[3 * H:] *= 2.0
    w4bx = np.zeros((din + 1, G), np.float32)
    w4bx[0:din] = wi
    w4bx[din] = b
    return wxs, whs, w4bx.astype(BF16)


def _prep_xin(x, cfg, core):
    """Per-core input tensor [2*TS, 17, 512] bf16:
    rows 0:8 x[t], 8:16 x[t-1] (lag), row 16 = in-range mask.
    Second half is the time-mirrored copy (index 2*TS-1-i) so a
    fwd+bwd chain pair can fetch both timesteps with one strided DMA."""
    tc, bh = core // cfg.n_bh, core % cfg.n_bh
    c0 = tc * cfg.chunk
    b0 = bh * cfg.CB
    xin = np.zeros((2 * cfg.TS, 17, cfg.CB), BF16)
    for i in range(cfg.TS):
        t = c0 - 4 * cfg.W + i
        if 0 <= t < cfg.T:
            xin[i, 0:8] = x[b0:b0 + cfg.CB, t, :].T.astype(BF16)
            if t - 1 >= 0:
                xin[i, 8:16] = x[b0:b0 + cfg.CB, t - 1, :].T.astype(BF16)
            xin[i, 16] = 1.0
    xin[cfg.TS:] = xin[:cfg.TS][::-1]
    return xin


# ------------------------- program builder -------------------------

def build_program(cfg):
    nc = bacc.Bacc(None, target_bir_lowering=False)
    NBT, TS, W, K = cfg.NBT, cfg.TS, cfg.W, cfg.K
    CB = cfg.CB

    xin = nc.declare_dram_parameter("xin", [2 * TS, 17, CB], BF, isOutput=False)
    zeros = nc.declare_dram_parameter("zeros", [48, CB], BF, isOutput=False)
    wxd = [nc.declare_dram_parameter(f"wx{l}", [_rowmap(l)[4], 8 * LAYERS[l][1]],
                                     BF, isOutput=False) for l in range(4)]
    whd = [nc.declare_dram_parameter(f"wh{l}", [2 * LAYERS[l][1], 8 * LAYERS[l][1]],
                                     BF, isOutput=False) for l in range(4)]
    w4b = nc.declare_dram_parameter("w4b", [21, 40], BF, isOutput=False)
    hf4out = nc.declare_dram_parameter("hf4out", [CB, 10], BF, isOutput=True)
    hb4out = nc.declare_dram_parameter("hb4out", [CB, 10], BF, isOutput=True)
    # rows 0:H = h_fwd(t), H:2H = h_bwd(t), row 2H = in-range mask (prepass);
    # t range [TS, 2TS) is the time-mirrored copy (see _prep_xin)
    lo = [nc.dram_tensor(f"lo{l}", [2 * TS, 2 * LAYERS[l][1] + 1, CB], BF)
          for l in range(3)]

    with TileContext(nc) as tc:
        with (
            tc.tile_pool(name="const", bufs=1) as constp,
            tc.tile_pool(name="stg", bufs=12) as stgp,
            tc.tile_pool(name="sig", bufs=6) as sigp,
            tc.tile_pool(name="gc", bufs=cfg.K + 2) as gcp,
            tc.tile_pool(name="pp", bufs=4) as ppp,
            tc.tile_pool(name="tch", bufs=6) as tcp,
            tc.tile_pool(name="hsb", bufs=2 * cfg.K + 2) as hsbp,
            tc.tile_pool(name="psg", bufs=3, space="PSUM") as psgp,
            tc.tile_pool(name="pst", bufs=2, space="PSUM") as pstp,
        ):
            ident = constp.tile([128, 128], BF, tag="ident")
            make_identity(nc, ident)
            wxt, wht = [], []
            for l in range(4):
                ktot = _rowmap(l)[4]
                H_ = LAYERS[l][1]
                G2 = 8 * H_
                t_ = constp.tile([ktot, G2], BF, tag=f"wx{l}")
                nc.sync.dma_start(t_[:, :], wxd[l][:, :])
                wxt.append(t_)
                t_ = constp.tile([2 * H_, G2], BF, tag=f"wh{l}")
                nc.sync.dma_start(t_[:, :], whd[l][:, :])
                wht.append(t_)
            w4bt = constp.tile([21, 40], BF, tag="w4b")
            nc.sync.dma_start(w4bt[:, :], w4b[:, :])
            # mask prepass: copy the in-range row into each layer-out buffer
            # (covers both the normal and mirrored halves)
            for l in range(3):
                H_ = LAYERS[l][1]
                nc.sync.dma_start(lo[l][:, 2 * H_:2 * H_ + 1, :],
                                  xin[:, 16:17, :])

            def run_pair(l, sub0_a, sub1_a, sub0_b, sub1_b, rev_b, S,
                         grab_hf4=False):
                """One pair of chains for layer l.
                Chain A: forward over t = sub0_a - W + s.
                Chain B: if rev_b, backward over t = sub1_b + W - 1 - s,
                else forward over t = sub0_b - W + s (layer 4).
                Valid outputs written to lo[l] when s >= W."""
                din, H = LAYERS[l]
                G = 4 * H
                xf0, onesf, xb0, onesb, ktot = _rowmap(l)

                def tA(s):
                    return sub0_a - W + s

                def tB(s):
                    return (sub1_b + W - 1 - s) if rev_b else (sub0_b - W + s)

                def fill_x(stg, s):
                    # one DMA fills [x_a; ones_a; x_b; ones_b] (contiguous
                    # dst rows; strided src over the two timesteps; chain B
                    # reads the mirrored half when it runs backward)
                    ta = tA(s)
                    tb = (2 * TS - 1 - tB(s)) if rev_b else tB(s)
                    d = tb - ta
                    assert d >= 1
                    src = (xin if l == 0 else lo[l - 1])[ta: ta + d + 1: d]
                    nc.gpsimd.dma_start(stg[:, :, :], src)

                stg = stgp.tile([ktot, NBT, 128], BF, tag="stg")
                fill_x(stg, 0)
                # zero initial state (written like the per-step h tiles)
                hsb = hsbp.tile([2 * H, NBT, 128], BF, tag="hsb")
                nc.sync.dma_start(hsb[:, :, :],
                                  zeros[0:2 * H, :].rearrange(
                                      "p (n b) -> p n b", n=NBT))
                # persistent per-pair tile; cols per (bt, ch): [gtil(H); C(H)]
                gc = gcp.tile([128, NBT, 2, 2 * H], BF, tag="gc")
                for s in range(S):
                    stg_n = stgp.tile([ktot, NBT, 128], BF, tag="stg")
                    if s + 1 < S:
                        fill_x(stg_n, s + 1)
                    # gates: x-part + recurrent part accumulate in PSUM
                    gps = psgp.tile([128, NBT, 256], FP32, tag="gps")
                    for bt in range(NBT):
                        nc.tensor.matmul(gps[:, bt, 0:2 * G],
                                         stg[:, bt, :], wxt[l][:, :],
                                         start=True, stop=False)
                        nc.tensor.matmul(gps[:, bt, 0:2 * G],
                                         hsb[:, bt, :], wht[l][:, :],
                                         start=False, stop=True)
                    sig = sigp.tile([128, NBT, 2, G], BF, tag="sig")
                    nc.scalar.activation(sig[:, :, :, :], gps[:, :, 0:2 * G],
                                         AF.Sigmoid)
                    # gtil(s) = 2*sigma(2g)-1 overwrites the dead gtil(s-1)
                    nc.vector.tensor_scalar(gc[:, :, :, 0:H],
                                            sig[:, :, :, 3 * H:4 * H],
                                            2.0, -1.0, ALU.mult, ALU.add)
                    if s == 0:
                        prod = ppp.tile([128, NBT, 2, H], BF, tag="pp0")
                        nc.vector.tensor_tensor(prod[:, :, :, :],
                                                sig[:, :, :, 0:H],
                                                gc[:, :, :, 0:H], ALU.mult)
                        nc.vector.tensor_copy(gc[:, :, :, H:2 * H],
                                              prod[:, :, :, :])
                    else:
                        prod = ppp.tile([128, NBT, 2, 2 * H], BF, tag="pp")
                        nc.vector.tensor_tensor(prod[:, :, :, :],
                                                sig[:, :, :, 0:2 * H],
                                                gc[:, :, :, :], ALU.mult)
                        nc.vector.tensor_tensor(gc[:, :, :, H:2 * H],
                                                prod[:, :, :, 0:H],
                                                prod[:, :, :, H:2 * H], ALU.add)
                    # tanh(C), h
                    tch = tcp.tile([128, NBT, 2, H], BF, tag="tch")
                    nc.scalar.activation(tch[:, :, :, :], gc[:, :, :, H:2 * H],
                                         AF.Tanh)
                    hs = tcp.tile([128, NBT, 2, H], BF, tag="hs")
                    nc.vector.tensor_tensor(hs[:, :, :, :],
                                            sig[:, :, :, 2 * H:3 * H],
                                            tch[:, :, :, :], ALU.mult)
                    if grab_hf4 and s == S - 1:
                        for bt in range(NBT):
                            nc.sync.dma_start(hf4out[bt * 128:(bt + 1) * 128, :],
                                              hs[:, bt, 1, :])
                    # transpose h back to matmul orientation
                    tps = pstp.tile([2 * H, NBT, 128], BF, tag="tps")
                    for bt in range(NBT):
                        nc.tensor.transpose(tps[:, bt, :], hs[:, bt, :, :],
                                            ident[:, :])
                    hsb = hsbp.tile([2 * H, NBT, 128], BF, tag="hsb")
                    nc.vector.tensor_copy(hsb[:, :, :], tps[:, :, :])
                    # emit valid outputs (layers 1..3 feed the next layer),
                    # to both the normal and mirrored time axes
                    if l < 3 and s >= W:
                        ta, tb = tA(s), tB(s)
                        nc.sync.dma_start(lo[l][ta, 0:H, :], hsb[0:H, :, :])
                        nc.sync.dma_start(lo[l][tb, H:2 * H, :],
                                          hsb[H:2 * H, :, :])
                        nc.sync.dma_start(lo[l][2 * TS - 1 - ta, 0:H, :],
                                          hsb[0:H, :, :])
                        nc.sync.dma_start(lo[l][2 * TS - 1 - tb, H:2 * H, :],
                                          hsb[H:2 * H, :, :])
                    stg = stg_n

            def run_phases():
                # ---- layers 1..3: K pairs of (fwd, bwd) sub-chunks ----
                for l in range(3):
                    v0, v1 = cfg.valid(l + 1)
                    span = v1 - v0
                    assert span % K == 0
                    sub = span // K
                    S = sub + W
                    for k in range(K):
                        a0 = v0 + k * sub
                        run_pair(l, a0, a0 + sub, a0, a0 + sub, True, S)

                # ---- layer 4: K pairs of (fwd, fwd) sub-chunks ----
                v0, v1 = cfg.valid(4)
                span = v1 - v0
                assert span % (2 * K) == 0
                sub = span // (2 * K)
                S4 = sub + W
                for k in range(K):
                    a0 = v0 + 2 * k * sub
                    b0 = v0 + (2 * k + 1) * sub
                    run_pair(3, a0, a0 + sub, b0, b0 + sub, False, S4,
                             grab_hf4=(k == K - 1))

                # ---- layer 4 backward: single step at the last timestep ----
                tlast = v1 - 1
                din, H = LAYERS[3]
                G = 4 * H
                stg1 = stgp.tile([21, NBT, 128], BF, tag="stg1")
                nc.sync.dma_start(stg1[:, :, :], lo[2][tlast, :, :])
                gps = psgp.tile([128, NBT, 256], FP32, tag="gps")
                for bt in range(NBT):
                    nc.tensor.matmul(gps[:, bt, 0:G], stg1[:, bt, :],
                                     w4bt[:, :], start=True, stop=True)
                sig = sigp.tile([128, NBT, G], BF, tag="sig4b")
                nc.scalar.activation(sig[:, :, :], gps[:, :, 0:G], AF.Sigmoid)
                gt = tcp.tile([128, NBT, H], BF, tag="gt4b")
                nc.vector.tensor_scalar(gt[:, :, :], sig[:, :, 3 * H:4 * H],
                                        2.0, -1.0, ALU.mult, ALU.add)
                cc = tcp.tile([128, NBT, H], BF, tag="cc4b")
                nc.vector.tensor_tensor(cc[:, :, :], sig[:, :, 0:H],
                                        gt[:, :, :], ALU.mult)
                tch = tcp.tile([128, NBT, H], BF, tag="tch4b")
                nc.scalar.activation(tch[:, :, :], cc[:, :, :], AF.Tanh)
                hb1 = tcp.tile([128, NBT, H], BF, tag="hb4b")
                nc.vector.tensor_tensor(hb1[:, :, :], sig[:, :, 2 * H:3 * H],
                                        tch[:, :, :], ALU.mult)
                for bt in range(NBT):
                    nc.sync.dma_start(hb4out[bt * 128:(bt + 1) * 128, :],
                                      hb1[:, bt, :])

            # cfg.reps > 1 repeats the whole computation (identical results)
            # so device time = (wall[reps=n] - wall[reps=1]) / (n - 1).
            for _rep in range(cfg.reps):
                run_phases()
    nc.compile()
    return nc


# ------------------------- entry point -------------------------

_CACHE = {}

def _get_program(cfg):
    key = (cfg.T, cfg.B, cfg.W, cfg.K, cfg.reps)
    if key not in _CACHE:
        _CACHE[key] = build_program(cfg)
    return _CACHE[key]


def kernel(_cfg=None, _trace=False, **inputs):
    from concourse.bass_utils import run_bass_kernel_spmd

    cfg = _cfg or Cfg()
    x = np.asarray(inputs["x"])
    wxs, whs, w4bm = _prep_weights(inputs)
    nc = _get_program(cfg)

    in_maps = []
    for core in range(cfg.n_cores):
        m = {"xin": _prep_xin(x, cfg, core), "w4b": w4bm,
             "zeros": np.zeros((48, cfg.CB), BF16)}
        for l in range(4):
            m[f"wx{l}"] = wxs[l]
            m[f"wh{l}"] = whs[l]
        in_maps.append(m)

    import time
    t0 = time.perf_counter()
    res = run_bass_kernel_spmd(nc, in_maps, list(range(cfg.n_cores)),
                               trace=_trace)
    kernel.last_wall_s = time.perf_counter() - t0
    results = res.results
    kernel.last_exec_time_ns = res.exec_time_ns

    # gather: last time-chunk cores hold t = T-1
    h4 = np.zeros((cfg.B, 20), np.float32)
    for bh in range(cfg.n_bh):
        core = (cfg.n_tc - 1) * cfg.n_bh + bh
        b0 = bh * cfg.CB
        h4[b0:b0 + cfg.CB, 0:10] = results[core]["hf4out"].astype(np.float32)
        h4[b0:b0 + cfg.CB, 10:20] = results[core]["hb4out"].astype(np.float32)

    fc_w = np.asarray(inputs["fc_w"], np.float32)
    fc_b = np.asarray(inputs["fc_b"], np.float32)
    z = h4 @ fc_w.T + fc_b
    return (1.0 / (1.0 + np.exp(-z))).astype(np.float32)
